# revision 20
# baseline (speedup 1.0000x reference)
"""BERT self-attention (B=4, S=1024, HID=1024, NH=16, HD=64) on 8 TRN2 NeuronCores.

Sharding: 8 shards = 4 batches x 2 head-halves. Core c handles batch c%4 and
heads [g*8, g*8+8) with g = c//4. Each core computes q/k/v projections for its
512 feature columns and full attention for its 8 heads; no collectives needed.
The host pre-transposes hidden_states / weights so the device never transposes.

Device-side layout:
  - q^T, k^T kept as [feat, seq]: scores are computed TRANSPOSED,
    s^T[keys, queries] = k^T.T @ q^T; exp needs no reduction first and the
    attention-mask bias is a per-partition ACT bias.
  - p~ = exp(s/8 + maskbias) goes from PSUM through ACT into SBUF fp16.
  - v gets a per-head ones column (v_aug [seq, 65]); ctx~^T = v_aug.T @ p~
    and row 64 of the result is the softmax denominator. ctx~/den ship to the
    host as fp16; the host computes num/den + bv (exact).

Schedule (v4): a budget-paced flat emission. 32 scores units (it = head-pack
x query-half, jc pair), each emitting 2 row-tiled score pairs (clustered so
the PE tile-config switch is paid once) + 2 biased exps. Between units the
PE is topped up to ~ACT-rate with, in priority order, deadline-due projection
fillers, backlogged ctx quads (lag >= 4 units behind their exp), and optional
fillers. This keeps ACT continuously fed (it is the slower steady stream),
keeps the PE dense, and leaves only a tiny ctx tail after the last exp.
Input DMA: issue begins only after the ~7us NEFF preamble and each HWDGE ring
stalls at ~5 outstanding issues, so the two rings are packed so that the
first qk0 matmul gates on ring-slot #1 of each, and later hsT chunks land
just ahead of their consumer; wv rides the gpsimd SWDGE queue.
"""
import os
import sys
from contextlib import ExitStack

for _p in ("/root/.axon_site/_ro/trn_rl_repo", "/opt/trn_rl_repo"):
    if os.path.isdir(_p) and _p not in sys.path:
        sys.path.append(_p)

import numpy as np
import concourse.bacc as bacc
import concourse.mybir as mybir
from concourse import tile
from concourse.bass_utils import run_bass_kernel_spmd

B, S, HID, NH, HD = 4, 1024, 1024, 16, 64
NCORES = 8
FSH = 512  # feature columns per core = 8 heads * 64
HC = 8  # hid contraction chunks of 128
JC = 8  # key/seq chunks of 128
SC = 2  # seq chunks of 512 (queries / moving dim)
FC = 4  # feature chunks of 128
NHL = 8  # local heads per core
NU = 32  # scores units: 8 its x 4 jc-pairs
CTX_MIN_LAG = 4  # ctx quad (it,q) may not be emitted before unit 4it+q+LAG

F32 = mybir.dt.float32
F16 = mybir.dt.float16
EXP = mybir.ActivationFunctionType.Exp

# emission cost estimates (us) for the budget pacer
C_SCORES = 0.66  # 2 row-tiled pairs + config switch
C_CTX = 0.87  # 4 ctx matmuls (2 jc x 2 heads)
C_VPIECE = 1.73  # 8-matmul v chain
C_QKHALF = 0.87  # 4-matmul q/k half chain
TARGET = 2.42  # per-unit PE budget ~ ACT drain rate (2x1067ns + margin)


def _build_nc():
    nc = bacc.Bacc(None, target_bir_lowering=False, debug=False)

    # pro: single prologue blob so ONE ring-slot-#1 DMA delivers everything
    # the qk0 chains and early ACT need: wk fc0 | wq fc0 | aux (bq|bk|maskbias
    # fp32 as 32 f16 columns) | hsT hc0
    PRO_N = 1024 + 1024 + 32 + 1024
    pro = nc.declare_dram_parameter("pro", [128, PRO_N], F16, isOutput=False)
    hsT = nc.declare_dram_parameter("hsT", [128, HC, S], F16, isOutput=False)
    wqT = nc.declare_dram_parameter("wqT", [128, FC, HC, 128], F16, isOutput=False)
    wkT = nc.declare_dram_parameter("wkT", [128, FC, HC, 128], F16, isOutput=False)
    wvT = nc.declare_dram_parameter("wvT", [128, HC, FSH], F16, isOutput=False)
    out = nc.declare_dram_parameter("out", [FC, SC, HD + 1, 2, 512], F16, isOutput=True)

    with tile.TileContext(nc) as tc, ExitStack() as ctx:
        ctx.enter_context(
            nc.allow_low_precision(reason="fp16 data/staging; fp32 accumulate")
        )
        const = ctx.enter_context(tc.tile_pool(name="const", bufs=1))

        pro_sb = const.tile([128, PRO_N], F16, tag="pro")
        wk0_v = pro_sb[:, 0:1024].rearrange("p (h c) -> p h c", h=HC)
        wq0_v = pro_sb[:, 1024:2048].rearrange("p (h c) -> p h c", h=HC)
        auxv = pro_sb[:, 2048:2080].bitcast(F32)  # [128, 16] fp32
        bq_sb = auxv[:, 0:FC]
        bk_sb = auxv[:, FC : 2 * FC]
        mb_sb = auxv[:, 2 * FC : 2 * FC + JC]
        hs0_v = pro_sb[:, 2080 : 2080 + S]

        hsT_sb = const.tile([128, HC, S], F16, tag="hsT")  # hc0 unused
        wq_sb = const.tile([128, FC, HC, 128], F16, tag="wq")  # fc0 unused
        wk_sb = const.tile([128, FC, HC, 128], F16, tag="wk")
        wv_sb = const.tile([128, HC, FSH], F16, tag="wv")

        def hs_v(hc, lo, hi):
            return hs0_v[:, lo:hi] if hc == 0 else hsT_sb[:, hc, lo:hi]

        # Input DMA. Issue begins only after the ~7us NEFF preamble; a ring's
        # transfers serialize in issue order and completions surface slowly,
        # so ring slot #1 is ONE blob carrying everything the qk0 chains and
        # early ACT gate on; later hsT slabs land just ahead of consumption;
        # trailing dummy reads keep the ring advancing so earlier slots'
        # completions surface promptly. wv and the later q/k feature packs
        # (first needed ~40us in) ride the gpsimd SWDGE / scalar rings.
        dscr = const.tile([128, 2, 8], F16, tag="dscr")
        nc.sync.dma_start(pro_sb[:], pro[:])
        nc.sync.dma_start(hsT_sb[:, 1:4, :], hsT[:, 1:4, :])
        nc.sync.dma_start(hsT_sb[:, 4:HC, :], hsT[:, 4:HC, :])
        nc.sync.dma_start(dscr[:, 0], pro[:, 0:8])
        nc.sync.dma_start(dscr[:, 1], pro[:, 0:8])
        HH = HC // 2
        nc.gpsimd.dma_start(wv_sb[:, 0:HH, :], wvT[:, 0:HH, :])
        nc.gpsimd.dma_start(wv_sb[:, HH:HC, :], wvT[:, HH:HC, :])
        for fc in range(1, FC):
            nc.scalar.dma_start(wq_sb[:, fc], wqT[:, fc])
            nc.scalar.dma_start(wk_sb[:, fc], wkT[:, fc])

        qT_sb = const.tile([128, FC, S], F16, tag="qT")
        kT_sb = const.tile([128, FC, S], F16, tag="kT")
        # v with per-head ones column: [seq_part, jc, head, 64 v + 1 one];
        # only the ones column needs initializing
        v_sb = const.tile([128, JC, NHL, HD + 1], F16, tag="v")
        nc.vector.memset(v_sb[:, :, :, HD : HD + 1], 1.0)

        # PE warm-up: a few dummy matmuls right at engine start open the HAM
        # activity window while the first input DMA finishes, so the qk0
        # chains hit the 2.4 GHz un-throttle ~2us sooner. Results unused.
        warm_sb = const.tile([128, 512], F16, tag="warm")
        nc.vector.memset(warm_sb[:], 0.01)
        with tc.tile_pool(name="ps_warm", bufs=1, space="PSUM") as ps_w:
            wps = ps_w.tile([128, 512], F32, tag="wps", name="wps")
            for _ in range(7):
                nc.tensor.matmul(
                    wps[:], warm_sb[:, 0:128], warm_sb[:], start=True, stop=True
                )

        # ---- qk0 prologue: the 3 fc0 chains scores(unit 0) gates on ----
        # (k sc0, k sc1, q sc0); hc-outer so each matmul gates only on its
        # own hsT chunk. These ~24 matmuls also warm the HAM clock gate.
        with tc.tile_pool(name="ps_p0", bufs=3, space="PSUM") as ps_p0:
            qk0 = [
                (ps_p0.tile([128, 512], F32, tag="p0", name=f"p0{n}"), w, b, d, sc)
                for n, (w, b, d, sc) in enumerate(
                    [
                        (wk0_v, bk_sb, kT_sb, 0),
                        (wk0_v, bk_sb, kT_sb, 1),
                        (wq0_v, bq_sb, qT_sb, 0),
                    ]
                )
            ]
            for hc in range(HC):
                for ps, w_v, b_sb, dst, sc in qk0:
                    nc.tensor.matmul(
                        ps[:],
                        w_v[:, hc, :],
                        hs_v(hc, sc * 512, (sc + 1) * 512),
                        start=(hc == 0),
                        stop=(hc == HC - 1),
                    )
            for ps, w_v, b_sb, dst, sc in qk0:
                nc.vector.tensor_scalar_add(
                    dst[:, 0, sc * 512 : (sc + 1) * 512], ps[:], b_sb[:, 0:1]
                )

        ps_s = ctx.enter_context(tc.tile_pool(name="ps_s", bufs=2, space="PSUM"))
        ps_c = ctx.enter_context(tc.tile_pool(name="ps_c", bufs=2, space="PSUM"))
        ps_p = ctx.enter_context(tc.tile_pool(name="ps_p", bufs=2, space="PSUM"))
        p_pool = ctx.enter_context(tc.tile_pool(name="p", bufs=3))
        ob_pool = ctx.enter_context(tc.tile_pool(name="ob", bufs=2))

        def v_piece(jc):
            """v projection chunk jc: v[seq 128, feat 512] (no bias)."""
            ps = ps_p.tile([128, 512], F32, tag="pp", name=f"ppv{jc}")
            for hc in range(HC):
                nc.tensor.matmul(
                    ps[:],
                    hs_v(hc, jc * 128, (jc + 1) * 128),
                    wv_sb[:, hc, :],
                    start=(hc == 0),
                    stop=(hc == HC - 1),
                )
            nc.vector.tensor_copy(
                v_sb[:, jc, :, 0:HD], ps[:].rearrange("p (h d) -> p h d", h=NHL)
            )

        qk_open = {}

        def qk_piece_half(fc, which, sc, half):
            w_sb, b_sb, dst = ((wq_sb, bq_sb, qT_sb), (wk_sb, bk_sb, kT_sb))[which]
            if half == 0:
                ps = ps_p.tile([128, 512], F32, tag="pp", name=f"pp{fc}{which}{sc}")
                qk_open[(fc, which, sc)] = ps
            else:
                ps = qk_open.pop((fc, which, sc))
            for hc in range(half * 4, half * 4 + 4):
                if fc == 0:
                    w_v = (wq0_v, wk0_v)[which][:, hc, :]
                else:
                    w_v = w_sb[:, fc, hc, :]
                nc.tensor.matmul(
                    ps[:],
                    w_v,
                    hs_v(hc, sc * 512, (sc + 1) * 512),
                    start=(hc == 0),
                    stop=(hc == HC - 1),
                )
            if half == 1:
                nc.vector.tensor_scalar_add(
                    dst[:, fc, sc * 512 : (sc + 1) * 512], ps[:], b_sb[:, fc : fc + 1]
                )

        def emit_scores_jc(g2, i, jc, ptb):
            """One key-chunk of scores + exp. Two heads row-tiled (K=64) into
            one [128,1024] PSUM tile; one N=1024 biased exp."""
            ps = ps_s.tile([128, 1024], F32, tag="ss", name=f"ss{jc}")
            for hh in range(2):
                lo = hh * 64
                nc.tensor.matmul(
                    ps[:, hh * 512 : (hh + 1) * 512],
                    kT_sb[lo : lo + 64, g2, jc * 128 : (jc + 1) * 128],
                    qT_sb[lo : lo + 64, g2, i * 512 : (i + 1) * 512],
                    start=True,
                    stop=True,
                    tile_position=(lo, 0),
                )
            nc.scalar.activation(
                ptb[:, :, jc, :],
                ps[:].rearrange("p (a b) -> p a b", a=2),
                EXP,
                bias=mb_sb[:, jc : jc + 1],
                scale=0.125,
            )

        def emit_ctx_jc(pcs, g2, jc, ptb):
            for hh in range(2):
                nc.tensor.matmul(
                    pcs[hh][:],
                    v_sb[:, jc, 2 * g2 + hh, :],
                    ptb[:, hh, jc, :],
                    start=(jc == 0),
                    stop=(jc == JC - 1),
                )

        # ---- budget-paced flat emission ----
        # filler pieces in dependency-safe order with emission deadlines
        # (unit index BEFORE which the piece must have been emitted):
        # v[jc] before ctx(0, jc//2) which can appear from unit 4+jc//2;
        # fc pack p: k + q-sc0 before scores(it=2p) at unit 8p; q-sc1
        # before scores(2p+1) at unit 8p+4.
        filler_list = (
            [("v", jc, 4 + jc // 2, C_VPIECE) for jc in range(4)]
            + [("qk", (0, 0, 1, h), 4, C_QKHALF) for h in (0, 1)]
            + [("v", jc, 4 + jc // 2, C_VPIECE) for jc in range(4, JC)]
            + [("qk", (1, 1, 0, h), 8, C_QKHALF) for h in (0, 1)]
            + [("qk", (1, 1, 1, h), 8, C_QKHALF) for h in (0, 1)]
            + [("qk", (1, 0, 0, h), 8, C_QKHALF) for h in (0, 1)]
            + [("qk", (1, 0, 1, h), 12, C_QKHALF) for h in (0, 1)]
            + [("qk", (2, 1, 0, h), 16, C_QKHALF) for h in (0, 1)]
            + [("qk", (2, 1, 1, h), 16, C_QKHALF) for h in (0, 1)]
            + [("qk", (2, 0, 0, h), 16, C_QKHALF) for h in (0, 1)]
            + [("qk", (2, 0, 1, h), 20, C_QKHALF) for h in (0, 1)]
            + [("qk", (3, 1, 0, h), 24, C_QKHALF) for h in (0, 1)]
            + [("qk", (3, 1, 1, h), 24, C_QKHALF) for h in (0, 1)]
            + [("qk", (3, 0, 0, h), 24, C_QKHALF) for h in (0, 1)]
            + [("qk", (3, 0, 1, h), 28, C_QKHALF) for h in (0, 1)]
        )
        fi = 0  # next filler
        v_done = 0  # count of emitted v pieces

        ptbs = {}
        pcss = {}
        ctx_next = 0  # next ctx quad (linear index 4*it+q) to emit
        ctx_done_it = -1  # highest it whose ctx fully emitted

        def emit_filler():
            nonlocal fi, v_done
            kind, arg, _, cost = filler_list[fi]
            fi += 1
            if kind == "v":
                v_piece(arg)
                v_done += 1
            else:
                qk_piece_half(*arg)
            return cost

        def emit_ctx_quad():
            nonlocal ctx_next, ctx_done_it
            itc, qc = ctx_next // 4, ctx_next % 4
            ctx_next += 1
            g2c, ic = itc // 2, itc % 2
            if qc == 0:
                pcss[itc] = [
                    ps_c.tile([HD + 1, 512], F32, tag="cc", name=f"cc{itc}{hh}")
                    for hh in (0, 1)
                ]
            ptb = ptbs[itc]
            emit_ctx_jc(pcss[itc], g2c, 2 * qc, ptb)
            emit_ctx_jc(pcss[itc], g2c, 2 * qc + 1, ptb)
            if qc == 3:
                pcs = pcss.pop(itc)
                ptbs.pop(itc)
                ctx_done_it = itc
                ob = ob_pool.tile([HD + 1, 2, 512], F16, tag="ob", name=f"ob{itc}")
                last = itc == NU // 4 - 1
                for hh in range(2):
                    # final it: second copy on the (now idle) scalar engine so
                    # the two tail copies run in parallel; mid-stream copies
                    # stay on DVE (ACT's FIFO must not block on ctx waits)
                    if last and hh == 1:
                        nc.scalar.copy(ob[:, hh, :], pcs[hh][:])
                    else:
                        nc.vector.tensor_copy(ob[:, hh, :], pcs[hh][:])
                    eng = nc.sync if hh == 0 else nc.scalar
                    eng.dma_start(out[g2c, ic, :, hh, :], ob[:, hh, :])

        def ctx_ready(u):
            if ctx_next >= 4 * (NU // 4):
                return False
            itc, qc = ctx_next // 4, ctx_next % 4
            if 4 * itc + qc + CTX_MIN_LAG > u:
                return False  # exp not safely complete yet
            return v_done == JC or itc > 0 or 2 * qc + 1 < v_done

        for u in range(NU):
            it, q = u // 4, u % 4
            g2, i = it // 2, it % 2
            if q == 0:
                # ptb reuse (bufs=3) requires ctx(it-3) fully emitted
                while it >= 3 and ctx_done_it < it - 3:
                    emit_ctx_quad()
                ptbs[it] = p_pool.tile([128, 2, JC, 512], F16, tag="pt", name=f"pt{it}")
            emit_scores_jc(g2, i, 2 * q, ptbs[it])
            emit_scores_jc(g2, i, 2 * q + 1, ptbs[it])
            budget = TARGET - C_SCORES
            # deadline-due fillers (always emitted, budget or not)
            while fi < len(filler_list) and filler_list[fi][2] <= u + 1:
                budget -= emit_filler()
            # top up: ctx backlog first, then optional fillers
            while budget > 0:
                if ctx_ready(u):
                    emit_ctx_quad()
                    budget -= C_CTX
                elif fi < len(filler_list):
                    budget -= emit_filler()
                else:
                    break
        while fi < len(filler_list):
            emit_filler()
        while ctx_next < NU:
            emit_ctx_quad()

    nc.compile()
    return nc


_NC = None


def _get_nc():
    global _NC
    if _NC is None:
        _NC = _build_nc()
    return _NC


# test-harness knobs (ignored in normal grading use)
TRACE = False
TRACE_DIR = None
LAST_RESULT = None


def _pack(mT):
    """[1024, N] contraction-major -> [128, 8, N] partition-major fp16 so one
    DMA moves contiguous bytes per partition (big DMA packets)."""
    n = mT.shape[1]
    return np.ascontiguousarray(
        mT.reshape(HC, 128, n).transpose(1, 0, 2)
    ).astype(np.float16)


def _pack_w(mT):
    """[1024, 512] -> [128, FC, HC, 128] fp16: fc-major so per-fc DMA chunks
    are contiguous and each matmul's stationary slice is [128, 128]."""
    return np.ascontiguousarray(
        mT.reshape(HC, 128, FC, 128).transpose(1, 2, 0, 3)
    ).astype(np.float16)


def _pack_core(hs, mask, Wq, bq, Wk, bk, Wv, b, sl):
    """Per-core input map, incl. the single-DMA prologue blob."""
    hsT = _pack(hs[b].T)
    wqT = _pack_w(Wq[sl, :].T)
    wkT = _pack_w(Wk[sl, :].T)
    aux = np.ascontiguousarray(
        np.concatenate(
            [
                bq[sl].reshape(FC, 128).T,
                bk[sl].reshape(FC, 128).T,
                ((mask[b, 0, 0, :] - 1.0) * 1.0e6).reshape(JC, 128).T,
            ],
            axis=1,
        ),
        dtype=np.float32,
    )
    pro = np.concatenate(
        [
            wkT[:, 0].reshape(128, HC * 128),
            wqT[:, 0].reshape(128, HC * 128),
            aux.view(np.float16),
            hsT[:, 0, :],
        ],
        axis=1,
    )
    return {
        "pro": np.ascontiguousarray(pro),
        "hsT": hsT,
        "wqT": wqT,
        "wkT": wkT,
        "wvT": _pack(Wv[sl, :].T),
    }


def kernel(hidden_states, attention_mask, Wq, bq, Wk, bk, Wv, bv):
    global LAST_RESULT
    hs = np.asarray(hidden_states, dtype=np.float32)
    mask = np.asarray(attention_mask, dtype=np.float32)
    Wq = np.asarray(Wq, dtype=np.float32)
    Wk = np.asarray(Wk, dtype=np.float32)
    Wv = np.asarray(Wv, dtype=np.float32)
    bq = np.asarray(bq, dtype=np.float32)
    bk = np.asarray(bk, dtype=np.float32)
    bv = np.asarray(bv, dtype=np.float32)

    in_maps = []
    for c in range(NCORES):
        b, g = c % B, c // B
        sl = slice(g * FSH, (g + 1) * FSH)
        in_maps.append(_pack_core(hs, mask, Wq, bq, Wk, bk, Wv, b, sl))

    nc = _get_nc()
    kw = {}
    if TRACE:
        kw = {"trace": True, "tmpdir": TRACE_DIR}
    res = run_bass_kernel_spmd(nc, in_maps, list(range(NCORES)), **kw)
    LAST_RESULT = res

    full = np.empty((B, S, HID), dtype=np.float32)
    for c in range(NCORES):
        b, g = c % B, c // B
        o = res.results[c]["out"].astype(np.float32)  # [FC, SC, 65, 2, 512]
        num = o[:, :, 0:HD, :, :]  # [g2, i, d, hh, q]
        den = o[:, :, HD : HD + 1, :, :]  # [g2, i, 1, hh, q]
        ctx = num / den
        # -> [S=(i,q), F=(g2,hh,d)]
        blk = ctx.transpose(1, 4, 0, 3, 2).reshape(S, FSH) + bv[g * FSH : (g + 1) * FSH]
        full[b, :, g * FSH : (g + 1) * FSH] = blk
    return full
